# revision 20
# baseline (speedup 1.0000x reference)
"""Trainium2 Bass kernel for GRU decoder layer (teacher forcing).

Reference computation (per batch row b, seq len T):
    emb_y = emb[y]                               [B,T,EMB]
    xs    = concat([emb_y, tile(enc_out)], -1)   [B,T,EMB+H]
    mx    = xs @ W_in + b_in                     [B,T,3H]
    per step t: mh = h @ U + b_rec
        z = sig(mx_z + mh_z); r = sig(mx_r + mh_r)
        hh = tanh(mx_h + r * mh_h)
        h  = z*h + (1-z)*hh   (frozen when t >= mask[b] via z:=1 clamp, the
                               +40 z-logit push makes sigmoid exactly 1.0f)
    logits = hs @ Wo + bo, zeroed where t >= mask[b]

Sharding: pure data-parallel over batch across 8 cores (4 rows each), no
collectives. Token order within a core: i = b*T + t. All matmuls run as
float32r (full-rate fp32 PE mode). The recurrence uses 4 concurrent PE
column-group streams (strip bases 0/32/64/96); per-step x-contributions are
injected into PSUM with selector matmuls from a strip-aligned "spread" copy
of mx, so the gate math needs no partition-misaligned reads.
"""

import sys

sys.path.insert(0, "/opt/trn_rl_repo")

import numpy as np

import concourse.bass as bass
import concourse.tile as tile
from concourse import bacc, mybir
from concourse.bass_utils import run_bass_kernel_spmd

F32 = mybir.dt.float32
F32R = mybir.dt.float32r
I32 = mybir.dt.int32
I16 = mybir.dt.int16
BF16 = mybir.dt.bfloat16
ADD = mybir.AluOpType.add
SUB = mybir.AluOpType.subtract
MULT = mybir.AluOpType.mult

# Full-size problem constants (hardcoded per harness contract)
VOCAB = 32000
EMB = 512
H = 1024
B = 32
T = 128
IN_DIM = EMB + H
N_CORES = 8
BS = B // N_CORES          # 4 batch rows per core
NG = 4                     # partition strip groups (bases 0,32,64,96)
GC = H // NG               # 256 cols of each gate per group
KC = H // 128              # 8 contraction chunks for H
EC = EMB // 128            # 4 contraction chunks for EMB


def r32(ap):
    """bitcast an fp32 AP to float32r for full-rate PE matmul"""
    return ap.bitcast(F32R)


def build_kernel(T_=T, VOCAB_=VOCAB, VBLK=500):
    NTOK = BS * T_
    NVB = VOCAB_ // VBLK
    assert T_ == T and VOCAB_ % VBLK == 0
    H3 = 3 * H

    nc = bacc.Bacc("TRN2", target_bir_lowering=False, debug=False)

    enc_out = nc.declare_dram_parameter("enc_out", [BS, H], F32, isOutput=False)
    enc_st = nc.declare_dram_parameter("enc_st", [BS, H], F32, isOutput=False)
    y_in = nc.declare_dram_parameter("y", [BS, T_], I32, isOutput=False)
    mask_in = nc.declare_dram_parameter("mask", [BS], I32, isOutput=False)
    emb = nc.declare_dram_parameter("emb", [VOCAB_, EMB], F32, isOutput=False)
    w_in = nc.declare_dram_parameter("w_in", [IN_DIM, H3], F32R, isOutput=False)
    b_in = nc.declare_dram_parameter("b_in", [1, H3], F32R, isOutput=False)
    u_w = nc.declare_dram_parameter("u_w", [H, H3], F32, isOutput=False)
    b_rec = nc.declare_dram_parameter("b_rec", [1, H3], F32R, isOutput=False)
    wo = nc.declare_dram_parameter("wo", [H, VOCAB_], F32R, isOutput=False)
    bo = nc.declare_dram_parameter("bo", [1, VOCAB_], F32, isOutput=False)
    id4 = nc.declare_dram_parameter("id4", [4, 4], F32, isOutput=False)
    id128 = nc.declare_dram_parameter("id128", [128, 128], F32, isOutput=False)
    sels = nc.declare_dram_parameter("sels", [128, 8, 4], BF16, isOutput=False)
    onesc = nc.declare_dram_parameter("onesc", [128, 128], F32R, isOutput=False)

    out = nc.declare_dram_parameter("out", [BS, T_, VOCAB_], F32, isOutput=True)

    # DRAM scratch for MX re-layout roundtrip: [t, b, 3H] with permuted cols:
    # [0:H] = h-gate; [H + 512*g + 256*q : +256] = gate q (z=0, r=1) block g
    mx_dram = nc.dram_tensor("mx_scratch", [T_, BS, H3], F32)

    with tile.TileContext(nc) as tc:
        with (
            tc.tile_pool(name="persist", bufs=1) as persist,
            tc.tile_pool(name="uw", bufs=1) as upool,
            tc.tile_pool(name="state", bufs=1) as state,
        ):
            # ---------------- constants + small inputs ----------------
            id4_sb = persist.tile([4, 4], F32)
            nc.sync.dma_start(out=id4_sb, in_=id4[:])
            id128_sb = persist.tile([128, 128], F32)
            nc.sync.dma_start(out=id128_sb, in_=id128[:])
            sels_sb = persist.tile([128, 8, 4], BF16)
            nc.sync.dma_start(out=sels_sb, in_=sels[:])
            ones_sb = persist.tile([128, 128], F32R)
            nc.sync.dma_start(out=ones_sb, in_=onesc[:])

            # masks: MASKR[p, b] = mask[b]; IOT[p, j] = p
            maskr = persist.tile([128, 4], I32)
            nc.sync.dma_start(
                out=maskr,
                in_=bass.AP(tensor=mask_in, offset=0, ap=[[0, 128], [1, 4]]),
            )
            iot = persist.tile([128, 4], I32)
            nc.gpsimd.iota(iot[:], pattern=[[0, 4]], base=0, channel_multiplier=1)
            # actm[p, b] = 1.0 if p < mask[b] else 0.0
            actm = persist.tile([128, 4], F32)
            nc.vector.tensor_tensor(
                actm[:], iot[:], maskr[:], mybir.AluOpType.is_lt
            )
            # clampv[p, b] = 40 * (1 - actm)
            clampv = persist.tile([128, 4], F32)
            nc.vector.tensor_scalar(clampv[:], actm[:], -40.0, 40.0, MULT, ADD)

            # persistent state tensors
            hst = state.tile([128, KC, NTOK], BF16)     # transposed h (mm lhsT)
            hstr = state.tile([128, KC, NTOK], F32R)    # transposed h (projection)
            h0t = state.tile([128, KC, 4], BF16)        # transposed initial state
            h_sb = state.tile([4, H], F32)             # row-form current h
            mx_spread = state.tile([128, T_ // 8, 512], BF16)  # z|r per strip

            # ---------------- phase 1: embed + MX ----------------
            with tc.tile_pool(name="ph1", bufs=1) as ph1:
                # token idx wrapped int16: idx[p, s] = token i = s*16+p,
                # i = b*T + t, so with s = b*(T/16)+u: t = u*16+p
                idx32 = ph1.tile([16, BS, T_ // 16], I32)
                nc.sync.dma_start(
                    out=idx32,
                    in_=y_in[:].rearrange("b (u p) -> p b u", p=16),
                )
                idx16 = ph1.tile([128, NTOK // 16], I16)
                nc.vector.memset(idx16[:], 0)
                nc.vector.tensor_copy(
                    idx16[0:16, :], idx32[:].rearrange("p b u -> p (b u)")
                )

                # gather: ey[p, c, :] = emb[token i = c*128+p]
                ey = ph1.tile([128, NTOK // 128, EMB], F32)
                nc.gpsimd.dma_gather(
                    out_ap=ey[:],
                    in_ap=emb[:],
                    idxs_ap=idx16[:],
                    num_idxs=NTOK,
                    num_idxs_reg=NTOK,
                    elem_size=EMB,
                )

                # transposes: EY -> EYT (emb on partitions); ctx, h0
                eyt = ph1.tile([128, EC, NTOK // 128, 128], F32R)
                ctx_sb = ph1.tile([4, H], F32)
                nc.sync.dma_start(out=ctx_sb, in_=enc_out[:])
                nc.sync.dma_start(out=h_sb, in_=enc_st[:])
                ctxt = ph1.tile([128, KC, 4], F32R)
                with tc.tile_pool(name="tpps", bufs=3, space="PSUM") as tpps:
                    for c in range(NTOK // 128):
                        for e in range(EC):
                            tp = tpps.tile([128, 128], F32, tag="tp")
                            nc.tensor.transpose(
                                tp[:], ey[:, c, 128 * e : 128 * (e + 1)],
                                id128_sb[:],
                            )
                            nc.vector.tensor_copy(eyt[:, e, c, :], tp[:])
                    for k in range(KC):
                        tp = tpps.tile([128, 128], F32, tag="tp")
                        nc.tensor.transpose(
                            tp[:, 0:4], ctx_sb[:, 128 * k : 128 * (k + 1)],
                            id4_sb[:],
                        )
                        nc.vector.tensor_copy(ctxt[:, k, :], tp[:, 0:4])
                        tp2 = tpps.tile([128, 128], F32, tag="tp")
                        nc.tensor.transpose(
                            tp2[:, 0:4], h_sb[:, 128 * k : 128 * (k + 1)],
                            id4_sb[:],
                        )
                        nc.vector.tensor_copy(h0t[:, k, :], tp2[:, 0:4])

                # MC = ctx @ W2 + b_in (+ b_rec on z,r cols)   [4, 3H]
                mcs = ph1.tile([128, H3], F32R)  # MC rows at strips 0/32/64/96
                with (
                    tc.tile_pool(name="mcp", bufs=4) as mcp,
                    tc.tile_pool(name="w2s", bufs=3) as w2s,
                    tc.tile_pool(name="bigps", bufs=1, space="PSUM") as bigps,
                ):
                    mcps = bigps.tile([4, H3], F32)
                    for n in range(H3 // 512):
                        ns = slice(512 * n, 512 * (n + 1))
                        for k in range(KC):
                            w2c = w2s.tile([128, 512], F32R, tag="w2")
                            nc.sync.dma_start(
                                out=w2c,
                                in_=w_in[EMB + 128 * k : EMB + 128 * (k + 1), ns],
                            )
                            nc.tensor.matmul(
                                mcps[:, ns], r32(ctxt[:, k, :]), r32(w2c[:]),
                                start=(k == 0), stop=False,
                            )
                        last = n >= 2 * H // 512  # h-cols get no b_rec
                        b_in_c = mcp.tile([1, 512], F32R, tag="bstr")
                        nc.sync.dma_start(out=b_in_c, in_=b_in[:, ns])
                        nc.tensor.matmul(
                            mcps[:, ns], r32(ones_sb[0:1, 0:4]),
                            r32(b_in_c[:]), start=False, stop=last,
                        )
                        if not last:
                            b_rec_c = mcp.tile([1, 512], F32R, tag="bstr")
                            nc.sync.dma_start(out=b_rec_c, in_=b_rec[:, ns])
                            nc.tensor.matmul(
                                mcps[:, ns], r32(ones_sb[0:1, 0:4]),
                                r32(b_rec_c[:]), start=False, stop=True,
                            )
                    mc_sb = mcp.tile([4, H3], F32, tag="mcsb", bufs=1)
                    nc.vector.tensor_copy(mc_sb[:], mcps[:])
                    for b in range(BS):
                        nc.sync.dma_start(
                            out=mcs[32 * b : 32 * b + 1, :],
                            in_=r32(mc_sb[b : b + 1, :]),
                        )

                # MX[t, b, :] = ey_b @ W1 + MC[b]  -> DRAM, streamed per (n, b)
                with (
                    tc.tile_pool(name="w1s", bufs=3) as w1s,
                    tc.tile_pool(name="mxo", bufs=4) as mxo,
                    tc.tile_pool(name="nps", bufs=4, space="PSUM") as nps,
                ):
                    for c in range(NTOK // 128):  # token tile (= batch row b)
                        for n in range(H3 // 512):
                            ns = slice(512 * n, 512 * (n + 1))
                            ps = nps.tile([128, 512], F32, tag="ps")
                            for e in range(EC):
                                w1c = w1s.tile([128, 512], F32R, tag="w1")
                                nc.sync.dma_start(
                                    out=w1c,
                                    in_=w_in[128 * e : 128 * (e + 1), ns],
                                )
                                nc.tensor.matmul(
                                    ps[:], r32(eyt[:, e, c, :]), r32(w1c[:]),
                                    start=(e == 0), stop=False,
                                )
                            nc.tensor.matmul(
                                ps[:], r32(ones_sb[32 * c : 32 * c + 1, :]),
                                r32(mcs[32 * c : 32 * c + 1, ns]),
                                start=False, stop=True,
                                tile_position=(32 * c, 0),
                            )
                            o = mxo.tile([128, 512], F32, tag="mxo")
                            if 512 * n < H:  # z cols: add inactive clamp
                                nc.vector.tensor_scalar(
                                    o[:], ps[:], clampv[:, c : c + 1], None, ADD
                                )
                            else:
                                nc.vector.tensor_copy(o[:], ps[:])
                            # permuted destination columns (see mx_dram note)
                            if n < 2 * H // 512:  # z or r: piece q, blocks 2n'
                                q, npr = divmod(n, H // 512)
                                dst = bass.AP(
                                    tensor=mx_dram,
                                    offset=c * H3 + H + 1024 * npr + 256 * q,
                                    ap=[[BS * H3, T_], [512, 2], [1, 256]],
                                )
                            else:
                                npr = n - 2 * H // 512
                                dst = bass.AP(
                                    tensor=mx_dram,
                                    offset=c * H3 + 512 * npr,
                                    ap=[[BS * H3, T_], [1, 512]],
                                )
                            nc.sync.dma_start(out=dst, in_=o[:])

                # spread z|r cols: partition 32g+4j+bb <- mx[8s+j, bb, zr of g]
                for g in range(NG):
                    for jj in range(8):
                        nc.gpsimd.dma_start(
                            out=mx_spread[32 * g + 4 * jj : 32 * g + 4 * jj + 4, :, :],
                            in_=(bass.AP(
                                tensor=mx_dram,
                                offset=jj * BS * H3 + H + 512 * g,
                                ap=[
                                    [H3, 4],             # bb
                                    [H3 * BS * 8, T_ // 8],  # s
                                    [1, 512],            # z|r of group g
                                ],
                            )),
                        )

                # U weights: 8 chunks of [128, 3H], resident for recurrence
                u_sb = []
                for k in range(KC):
                    t_ = upool.tile([128, H3], BF16, tag=f"u{k}")
                    nc.gpsimd.dma_start(out=t_, in_=u_w[128 * k : 128 * (k + 1), :])
                    u_sb.append(t_)

            # ---------------- phase 2: recurrence ----------------
            with (
                tc.tile_pool(name="rzr", bufs=2) as rzr,
                tc.tile_pool(name="r1", bufs=1) as r1,
                tc.tile_pool(name="mxhp", bufs=2) as mxhp,
                tc.tile_pool(name="recps", bufs=2, space="PSUM") as recps,
                tc.tile_pool(name="tps2", bufs=2, space="PSUM") as tps2,
            ):
                u3 = [
                    u_sb[k][:].rearrange("p (a c) -> p a c", c=GC)
                    for k in range(KC)
                ]
                for t in range(T_):
                    j, s = t % 8, t // 8
                    mxh = mxhp.tile([4, H], F32, tag="mxh")
                    nc.sync.dma_start(out=mxh, in_=mx_dram[t, :, 0:H])

                    ps = recps.tile([128, 768], F32, tag="ps")
                    if t == 0:
                        lhs = [h0t[:, k, :] for k in range(KC)]
                    else:
                        lhs = [
                            hst[:, k, t - 1 :: T_] for k in range(KC)
                        ]
                    # selector matmuls first (they open the accum groups)
                    for g in range(NG):
                        nc.tensor.matmul(
                            ps[32 * g : 32 * g + 4, 0:512],
                            sels_sb[32 * g : 32 * (g + 1), j, :],
                            mx_spread[32 * g : 32 * (g + 1), s, :],
                            start=True, stop=False,
                            tile_position=(32 * g, 32 * g),
                        )
                    # round-robin strips so the 4 column-group streams overlap
                    for k in range(KC):
                        for g in range(NG):
                            nc.tensor.matmul(
                                ps[32 * g : 32 * g + 4, 0:512], lhs[k],
                                u3[k][:, g : g + NG + 1 : NG, :],
                                start=False, stop=(k == KC - 1),
                                tile_position=(0, 32 * g),
                            )
                    for k in range(KC):
                        for g in range(NG):
                            nc.tensor.matmul(
                                ps[32 * g : 32 * g + 4, 512:768], lhs[k],
                                u3[k][:, 2 * NG + g, :],
                                start=(k == 0), stop=(k == KC - 1),
                                tile_position=(0, 32 * g),
                            )

                    zr = rzr.tile([4, 2 * H], F32, tag="zr")
                    zr3 = zr[:].rearrange("p (a c) -> p a c", c=GC)
                    rm = r1.tile([4, H], F32, tag="rm")
                    for g in range(NG):
                        gp = slice(32 * g, 32 * g + 4)
                        nc.scalar.activation(
                            out=zr3[:, g : g + NG + 1 : NG, :],
                            in_=ps[gp, 0:512],
                            func=mybir.ActivationFunctionType.Sigmoid,
                        )
                        nc.vector.tensor_tensor(
                            rm[:, GC * g : GC * (g + 1)],
                            zr[:, H + GC * g : H + GC * (g + 1)],
                            ps[gp, 512:768],
                            MULT,
                        )
                    a_t = r1.tile([4, H], F32, tag="a")
                    nc.vector.tensor_tensor(a_t[:], rm[:], mxh[:], ADD)
                    hh = r1.tile([4, H], F32, tag="hh")
                    nc.scalar.activation(
                        out=hh[:], in_=a_t[:],
                        func=mybir.ActivationFunctionType.Tanh,
                    )
                    d_t = r1.tile([4, H], F32, tag="d")
                    nc.gpsimd.tensor_tensor(d_t[:], h_sb[:], hh[:], SUB)
                    e_t = r1.tile([4, H], F32, tag="e")
                    nc.vector.tensor_tensor(e_t[:], zr[:, 0:H], d_t[:], MULT)
                    nc.gpsimd.tensor_tensor(h_sb[:], hh[:], e_t[:], ADD)

                    # transpose h -> hst[:, :, b*T + t]
                    tp = tps2.tile([128, KC, 4], F32, tag="tp2")
                    for k in range(KC):
                        nc.tensor.transpose(
                            tp[:, k, :], h_sb[:, 128 * k : 128 * (k + 1)],
                            id4_sb[:],
                        )
                    nc.vector.tensor_copy(hst[:, :, t::T_], tp[:])
                    nc.scalar.copy(hstr[:, :, t::T_], tp[:])

            # ---------------- phase 3: projection ----------------
            with (
                tc.tile_pool(name="wop", bufs=2) as wop,
                tc.tile_pool(name="post", bufs=4) as post,
                tc.tile_pool(name="borp", bufs=2) as borp,
                tc.tile_pool(name="prps", bufs=8, space="PSUM") as prps,
            ):
                wor = wo[:].rearrange("(k p) v -> p k v", p=128)
                for v in range(NVB):
                    vs = slice(VBLK * v, VBLK * (v + 1))
                    woc = wop.tile([128, KC, VBLK], F32R, tag="wo")
                    nc.sync.dma_start(out=woc, in_=wor[:, :, vs])
                    borr = borp.tile([128, VBLK], F32, tag="bor")
                    nc.sync.dma_start(
                        out=borr,
                        in_=bass.AP(
                            tensor=bo, offset=VBLK * v, ap=[[0, 128], [1, VBLK]]
                        ),
                    )
                    for b in range(BS):
                        pr = prps.tile([128, VBLK], F32, tag="pr")
                        for k in range(KC):
                            nc.tensor.matmul(
                                pr[:],
                                hstr[:, k, T_ * b : T_ * (b + 1)],
                                r32(woc[:, k, :]),
                                start=(k == 0), stop=(k == KC - 1),
                            )
                        o1 = post.tile([128, VBLK], F32, tag="o1")
                        nc.vector.tensor_tensor(o1[:], pr[:], borr[:], ADD)
                        o2 = post.tile([128, VBLK], F32, tag="o2")
                        nc.scalar.mul(o2[:], o1[:], actm[:, b : b + 1])
                        nc.scalar.dma_start(out=out[b, :, vs], in_=o2[:])

    nc.compile()
    return nc


_CACHED = {}


def _get_kernel():
    if "nc" not in _CACHED:
        _CACHED["nc"] = build_kernel()
    return _CACHED["nc"]


def host_consts():
    id4 = np.eye(4, dtype=np.float32)
    id128 = np.eye(128, dtype=np.float32)
    import ml_dtypes
    sels = np.zeros((128, 8, 4), dtype=ml_dtypes.bfloat16)
    for g in range(4):
        for j in range(8):
            for m in range(4):
                sels[32 * g + 4 * j + m, j, m] = 1.0
    onesc = np.ones((128, 128), dtype=np.float32)
    return {"id4": id4, "id128": id128, "sels": sels, "onesc": onesc}


def make_in_maps(
    encoder_outputs, encoder_state, y, mask, emb, W_in, b_in, U, b_rec, Wo, bo,
    n_cores=N_CORES,
):
    consts = host_consts()
    h3 = 3 * H
    in_maps = []
    bs = encoder_outputs.shape[0] // n_cores
    for c in range(n_cores):
        rows = slice(bs * c, bs * (c + 1))
        in_maps.append(
            {
                "enc_out": np.ascontiguousarray(encoder_outputs[rows], np.float32),
                "enc_st": np.ascontiguousarray(encoder_state[rows], np.float32),
                "y": np.ascontiguousarray(y[rows], np.int32),
                "mask": np.ascontiguousarray(mask[rows], np.int32),
                "emb": np.ascontiguousarray(emb, np.float32),
                "w_in": np.ascontiguousarray(W_in, np.float32),
                "b_in": np.ascontiguousarray(b_in, np.float32).reshape(1, h3),
                "u_w": np.ascontiguousarray(U, np.float32),
                "b_rec": np.ascontiguousarray(b_rec, np.float32).reshape(1, h3),
                "wo": np.ascontiguousarray(Wo, np.float32),
                "bo": np.ascontiguousarray(bo, np.float32).reshape(1, -1),
                **consts,
            }
        )
    return in_maps


def kernel(
    encoder_outputs, encoder_state, y, mask, emb, W_in, b_in, U, b_rec, Wo, bo
):
    if np.any(np.asarray(b_rec).reshape(-1)[2 * H :]):
        raise NotImplementedError("nonzero b_rec_h not supported")
    nc = _get_kernel()
    in_maps = make_in_maps(
        encoder_outputs, encoder_state, y, mask, emb, W_in, b_in, U, b_rec,
        Wo, bo,
    )
    res = run_bass_kernel_spmd(nc, in_maps, core_ids=list(range(N_CORES)))
    outs = [res.results[c]["out"] for c in range(N_CORES)]
    return np.concatenate(outs, axis=0).astype(np.float32)


# revision 21
# speedup vs baseline: 1.2060x; 1.2060x over previous
"""Trainium2 Bass kernel for GRU decoder layer (teacher forcing).

Reference computation (per batch row b, seq len T):
    emb_y = emb[y]                               [B,T,EMB]
    xs    = concat([emb_y, tile(enc_out)], -1)   [B,T,EMB+H]
    mx    = xs @ W_in + b_in                     [B,T,3H]
    per step t: mh = h @ U + b_rec
        z = sig(mx_z + mh_z); r = sig(mx_r + mh_r)
        hh = tanh(mx_h + r * mh_h)
        h  = z*h + (1-z)*hh   (frozen when t >= mask[b] via z:=1 clamp, the
                               +40 z-logit push makes sigmoid exactly 1.0f)
    logits = hs @ Wo + bo, zeroed where t >= mask[b]

Sharding: pure data-parallel over batch across 8 cores (4 rows each), no
collectives. Token order within a core: i = b*T + t. All matmuls run as
float32r (full-rate fp32 PE mode). The recurrence uses 4 concurrent PE
column-group streams (strip bases 0/32/64/96); per-step x-contributions are
injected into PSUM with selector matmuls from a strip-aligned "spread" copy
of mx, so the gate math needs no partition-misaligned reads.
"""

import sys

sys.path.insert(0, "/opt/trn_rl_repo")

import numpy as np

import concourse.bass as bass
import concourse.tile as tile
from concourse import bacc, mybir
from concourse.bass_utils import run_bass_kernel_spmd

F32 = mybir.dt.float32
F32R = mybir.dt.float32r
I32 = mybir.dt.int32
I16 = mybir.dt.int16
BF16 = mybir.dt.bfloat16
ADD = mybir.AluOpType.add
SUB = mybir.AluOpType.subtract
MULT = mybir.AluOpType.mult

# Full-size problem constants (hardcoded per harness contract)
VOCAB = 32000
EMB = 512
H = 1024
B = 32
T = 128
IN_DIM = EMB + H
N_CORES = 8
BS = B // N_CORES          # 4 batch rows per core
NG = 4                     # partition strip groups (bases 0,32,64,96)
GC = H // NG               # 256 cols of each gate per group
KC = H // 128              # 8 contraction chunks for H
EC = EMB // 128            # 4 contraction chunks for EMB


def r32(ap):
    """bitcast an fp32 AP to float32r for full-rate PE matmul"""
    return ap.bitcast(F32R)


def build_kernel(T_=T, VOCAB_=VOCAB, VBLK=500, f32r_mx=True, f32r_mc=False,
                 f32r_proj=True):
    DT_MX = F32R if f32r_mx else F32
    DT_MC = F32R if f32r_mc else F32
    DT_PROJ = F32R if f32r_proj else F32
    c_mx = r32 if f32r_mx else (lambda ap: ap)
    c_mc = r32 if f32r_mc else (lambda ap: ap)
    c_proj = r32 if f32r_proj else (lambda ap: ap)
    NTOK = BS * T_
    NVB = VOCAB_ // VBLK
    assert T_ == T and VOCAB_ % VBLK == 0
    H3 = 3 * H

    nc = bacc.Bacc("TRN2", target_bir_lowering=False, debug=False)

    enc_out = nc.declare_dram_parameter("enc_out", [BS, H], F32, isOutput=False)
    enc_st = nc.declare_dram_parameter("enc_st", [BS, H], F32, isOutput=False)
    y_in = nc.declare_dram_parameter("y", [BS, T_], I32, isOutput=False)
    mask_in = nc.declare_dram_parameter("mask", [BS], I32, isOutput=False)
    emb = nc.declare_dram_parameter("emb", [VOCAB_, EMB], F32, isOutput=False)
    w_in = nc.declare_dram_parameter("w_in", [IN_DIM, H3], F32, isOutput=False)
    b_in = nc.declare_dram_parameter("b_in", [1, H3], F32, isOutput=False)
    u_w = nc.declare_dram_parameter("u_w", [H, H3], F32, isOutput=False)
    b_rec = nc.declare_dram_parameter("b_rec", [1, H3], F32, isOutput=False)
    wo = nc.declare_dram_parameter("wo", [H, VOCAB_], F32, isOutput=False)
    bo = nc.declare_dram_parameter("bo", [1, VOCAB_], F32, isOutput=False)
    id4 = nc.declare_dram_parameter("id4", [4, 4], F32, isOutput=False)
    id128 = nc.declare_dram_parameter("id128", [128, 128], F32, isOutput=False)
    sels = nc.declare_dram_parameter("sels", [128, 8, 4], BF16, isOutput=False)
    onesc = nc.declare_dram_parameter("onesc", [128, 128], F32, isOutput=False)

    out = nc.declare_dram_parameter("out", [BS, T_, VOCAB_], F32, isOutput=True)

    # DRAM scratch for MX re-layout roundtrip: [t, b, 3H] with permuted cols:
    # [0:H] = h-gate; [H + 512*g + 256*q : +256] = gate q (z=0, r=1) block g
    mx_dram = nc.dram_tensor("mx_scratch", [T_, BS, H3], F32)

    with tile.TileContext(nc) as tc:
        with (
            tc.tile_pool(name="persist", bufs=1) as persist,
            tc.tile_pool(name="uw", bufs=1) as upool,
            tc.tile_pool(name="state", bufs=1) as state,
        ):
            # ---------------- constants + small inputs ----------------
            id4_sb = persist.tile([4, 4], F32)
            nc.sync.dma_start(out=id4_sb, in_=id4[:])
            id128_sb = persist.tile([128, 128], F32)
            nc.sync.dma_start(out=id128_sb, in_=id128[:])
            sels_sb = persist.tile([128, 8, 4], BF16)
            nc.sync.dma_start(out=sels_sb, in_=sels[:])
            ones_sb = persist.tile([128, 128], DT_MC)
            nc.sync.dma_start(out=ones_sb, in_=c_mc(onesc[:]))

            # masks: MASKR[p, b] = mask[b]; IOT[p, j] = p
            maskr = persist.tile([128, 4], I32)
            nc.sync.dma_start(
                out=maskr,
                in_=bass.AP(tensor=mask_in, offset=0, ap=[[0, 128], [1, 4]]),
            )
            iot = persist.tile([128, 4], I32)
            nc.gpsimd.iota(iot[:], pattern=[[0, 4]], base=0, channel_multiplier=1)
            # actm[p, b] = 1.0 if p < mask[b] else 0.0
            actm = persist.tile([128, 4], F32)
            nc.vector.tensor_tensor(
                actm[:], iot[:], maskr[:], mybir.AluOpType.is_lt
            )
            # clampv[p, b] = 40 * (1 - actm)
            clampv = persist.tile([128, 4], F32)
            nc.vector.tensor_scalar(clampv[:], actm[:], -40.0, 40.0, MULT, ADD)

            # persistent state tensors
            hst = state.tile([128, KC, NTOK], BF16)     # transposed h (mm lhsT)
            hstr = state.tile([128, KC, NTOK], DT_PROJ)  # transposed h (projection)
            h0t = state.tile([128, KC, 4], BF16)        # transposed initial state
            h_sb = state.tile([4, H], F32)             # row-form current h
            mx_spread = state.tile([128, T_ // 8, 512], BF16)  # z|r per strip

            # ---------------- phase 1: embed + MX ----------------
            with tc.tile_pool(name="ph1", bufs=1) as ph1:
                # token idx wrapped int16: idx[p, s] = token i = s*16+p,
                # i = b*T + t, so with s = b*(T/16)+u: t = u*16+p
                idx32 = ph1.tile([16, BS, T_ // 16], I32)
                nc.sync.dma_start(
                    out=idx32,
                    in_=y_in[:].rearrange("b (u p) -> p b u", p=16),
                )
                idx16 = ph1.tile([128, NTOK // 16], I16)
                nc.vector.memset(idx16[:], 0)
                nc.vector.tensor_copy(
                    idx16[0:16, :], idx32[:].rearrange("p b u -> p (b u)")
                )

                # gather: ey[p, c, :] = emb[token i = c*128+p]
                ey = ph1.tile([128, NTOK // 128, EMB], F32)
                nc.gpsimd.dma_gather(
                    out_ap=ey[:],
                    in_ap=emb[:],
                    idxs_ap=idx16[:],
                    num_idxs=NTOK,
                    num_idxs_reg=NTOK,
                    elem_size=EMB,
                )

                # transposes: EY -> EYT (emb on partitions); ctx, h0
                eyt = ph1.tile([128, EC, NTOK // 128, 128], DT_MX)
                ctx_sb = ph1.tile([4, H], F32)
                nc.sync.dma_start(out=ctx_sb, in_=enc_out[:])
                nc.sync.dma_start(out=h_sb, in_=enc_st[:])
                ctxt = ph1.tile([128, KC, 4], DT_MC)
                with tc.tile_pool(name="tpps", bufs=3, space="PSUM") as tpps:
                    for c in range(NTOK // 128):
                        for e in range(EC):
                            tp = tpps.tile([128, 128], F32, tag="tp")
                            nc.tensor.transpose(
                                tp[:], ey[:, c, 128 * e : 128 * (e + 1)],
                                id128_sb[:],
                            )
                            nc.vector.tensor_copy(eyt[:, e, c, :], tp[:])
                    for k in range(KC):
                        tp = tpps.tile([128, 128], F32, tag="tp")
                        nc.tensor.transpose(
                            tp[:, 0:4], ctx_sb[:, 128 * k : 128 * (k + 1)],
                            id4_sb[:],
                        )
                        nc.vector.tensor_copy(ctxt[:, k, :], tp[:, 0:4])
                        tp2 = tpps.tile([128, 128], F32, tag="tp")
                        nc.tensor.transpose(
                            tp2[:, 0:4], h_sb[:, 128 * k : 128 * (k + 1)],
                            id4_sb[:],
                        )
                        nc.vector.tensor_copy(h0t[:, k, :], tp2[:, 0:4])

                # MC = ctx @ W2 + b_in (+ b_rec on z,r cols)   [4, 3H]
                mcs = ph1.tile([128, H3], DT_MC)  # MC rows at strips 0/32/64/96
                with (
                    tc.tile_pool(name="mcp", bufs=4) as mcp,
                    tc.tile_pool(name="w2s", bufs=3) as w2s,
                    tc.tile_pool(name="bigps", bufs=1, space="PSUM") as bigps,
                ):
                    mcps = bigps.tile([4, H3], F32)
                    for n in range(H3 // 512):
                        ns = slice(512 * n, 512 * (n + 1))
                        for k in range(KC):
                            w2c = w2s.tile([128, 512], DT_MC, tag="w2")
                            nc.sync.dma_start(
                                out=w2c,
                                in_=c_mc(w_in[EMB + 128 * k : EMB + 128 * (k + 1), ns]),
                            )
                            nc.tensor.matmul(
                                mcps[:, ns], ctxt[:, k, :], w2c[:],
                                start=(k == 0), stop=False,
                            )
                        last = n >= 2 * H // 512  # h-cols get no b_rec
                        b_in_c = mcp.tile([1, 512], DT_MC, tag="bstr")
                        nc.sync.dma_start(out=b_in_c, in_=c_mc(b_in[:, ns]))
                        nc.tensor.matmul(
                            mcps[:, ns], ones_sb[0:1, 0:4],
                            b_in_c[:], start=False, stop=last,
                        )
                        if not last:
                            b_rec_c = mcp.tile([1, 512], DT_MC, tag="bstr")
                            nc.sync.dma_start(out=b_rec_c, in_=c_mc(b_rec[:, ns]))
                            nc.tensor.matmul(
                                mcps[:, ns], ones_sb[0:1, 0:4],
                                b_rec_c[:], start=False, stop=True,
                            )
                    mc_sb = mcp.tile([4, H3], F32, tag="mcsb", bufs=1)
                    nc.vector.tensor_copy(mc_sb[:], mcps[:])
                    for b in range(BS):
                        nc.sync.dma_start(
                            out=mcs[32 * b : 32 * b + 1, :],
                            in_=c_mc(mc_sb[b : b + 1, :]),
                        )

                # MX[t, b, :] = ey_b @ W1 + MC[b]  -> DRAM, streamed per (n, b)
                with (
                    tc.tile_pool(name="w1s", bufs=3) as w1s,
                    tc.tile_pool(name="mxo", bufs=4) as mxo,
                    tc.tile_pool(name="nps", bufs=4, space="PSUM") as nps,
                ):
                    for c in range(NTOK // 128):  # token tile (= batch row b)
                        for n in range(H3 // 512):
                            ns = slice(512 * n, 512 * (n + 1))
                            ps = nps.tile([128, 512], F32, tag="ps")
                            for e in range(EC):
                                w1c = w1s.tile([128, 512], DT_MX, tag="w1")
                                nc.sync.dma_start(
                                    out=w1c,
                                    in_=c_mx(w_in[128 * e : 128 * (e + 1), ns]),
                                )
                                nc.tensor.matmul(
                                    ps[:], eyt[:, e, c, :], w1c[:],
                                    start=(e == 0), stop=False,
                                )
                            nc.tensor.matmul(
                                ps[:], ones_sb[32 * c : 32 * c + 1, :],
                                mcs[32 * c : 32 * c + 1, ns],
                                start=False, stop=True,
                                tile_position=(32 * c, 0),
                            )
                            o = mxo.tile([128, 512], F32, tag="mxo")
                            if 512 * n < H:  # z cols: add inactive clamp
                                nc.vector.tensor_scalar(
                                    o[:], ps[:], clampv[:, c : c + 1], None, ADD
                                )
                            else:
                                nc.vector.tensor_copy(o[:], ps[:])
                            # permuted destination columns (see mx_dram note)
                            if n < 2 * H // 512:  # z or r: piece q, blocks 2n'
                                q, npr = divmod(n, H // 512)
                                dst = bass.AP(
                                    tensor=mx_dram,
                                    offset=c * H3 + H + 1024 * npr + 256 * q,
                                    ap=[[BS * H3, T_], [512, 2], [1, 256]],
                                )
                            else:
                                npr = n - 2 * H // 512
                                dst = bass.AP(
                                    tensor=mx_dram,
                                    offset=c * H3 + 512 * npr,
                                    ap=[[BS * H3, T_], [1, 512]],
                                )
                            nc.sync.dma_start(out=dst, in_=o[:])

                # spread z|r cols: partition 32g+4j+bb <- mx[8s+j, bb, zr of g]
                for g in range(NG):
                    for jj in range(8):
                        nc.gpsimd.dma_start(
                            out=mx_spread[32 * g + 4 * jj : 32 * g + 4 * jj + 4, :, :],
                            in_=(bass.AP(
                                tensor=mx_dram,
                                offset=jj * BS * H3 + H + 512 * g,
                                ap=[
                                    [H3, 4],             # bb
                                    [H3 * BS * 8, T_ // 8],  # s
                                    [1, 512],            # z|r of group g
                                ],
                            )),
                        )

                # U weights: 8 chunks of [128, 3H], resident for recurrence
                u_sb = []
                for k in range(KC):
                    t_ = upool.tile([128, H3], BF16, tag=f"u{k}")
                    nc.gpsimd.dma_start(out=t_, in_=u_w[128 * k : 128 * (k + 1), :])
                    u_sb.append(t_)

            # ---------------- phase 2: recurrence ----------------
            with (
                tc.tile_pool(name="rzr", bufs=2) as rzr,
                tc.tile_pool(name="r1", bufs=1) as r1,
                tc.tile_pool(name="mxhp", bufs=2) as mxhp,
                tc.tile_pool(name="recps", bufs=2, space="PSUM") as recps,
                tc.tile_pool(name="tps2", bufs=2, space="PSUM") as tps2,
            ):
                u3 = [
                    u_sb[k][:].rearrange("p (a c) -> p a c", c=GC)
                    for k in range(KC)
                ]
                for t in range(T_):
                    j, s = t % 8, t // 8
                    mxh = mxhp.tile([4, H], F32, tag="mxh")
                    nc.sync.dma_start(out=mxh, in_=mx_dram[t, :, 0:H])

                    ps = recps.tile([128, 768], F32, tag="ps")
                    if t == 0:
                        lhs = [h0t[:, k, :] for k in range(KC)]
                    else:
                        lhs = [
                            hst[:, k, t - 1 :: T_] for k in range(KC)
                        ]
                    # selector matmuls first (they open the accum groups)
                    for g in range(NG):
                        nc.tensor.matmul(
                            ps[32 * g : 32 * g + 4, 0:512],
                            sels_sb[32 * g : 32 * (g + 1), j, :],
                            mx_spread[32 * g : 32 * (g + 1), s, :],
                            start=True, stop=False,
                            tile_position=(32 * g, 32 * g),
                        )
                    # round-robin strips so the 4 column-group streams overlap
                    for k in range(KC):
                        for g in range(NG):
                            nc.tensor.matmul(
                                ps[32 * g : 32 * g + 4, 0:512], lhs[k],
                                u3[k][:, g : g + NG + 1 : NG, :],
                                start=False, stop=(k == KC - 1),
                                tile_position=(0, 32 * g),
                            )
                    for k in range(KC):
                        for g in range(NG):
                            nc.tensor.matmul(
                                ps[32 * g : 32 * g + 4, 512:768], lhs[k],
                                u3[k][:, 2 * NG + g, :],
                                start=(k == 0), stop=(k == KC - 1),
                                tile_position=(0, 32 * g),
                            )

                    zr = rzr.tile([4, 2 * H], F32, tag="zr")
                    zr3 = zr[:].rearrange("p (a c) -> p a c", c=GC)
                    rm = r1.tile([4, H], F32, tag="rm")
                    for g in range(NG):
                        gp = slice(32 * g, 32 * g + 4)
                        nc.scalar.activation(
                            out=zr3[:, g : g + NG + 1 : NG, :],
                            in_=ps[gp, 0:512],
                            func=mybir.ActivationFunctionType.Sigmoid,
                        )
                        nc.vector.tensor_tensor(
                            rm[:, GC * g : GC * (g + 1)],
                            zr[:, H + GC * g : H + GC * (g + 1)],
                            ps[gp, 512:768],
                            MULT,
                        )
                    a_t = r1.tile([4, H], F32, tag="a")
                    nc.vector.tensor_tensor(a_t[:], rm[:], mxh[:], ADD)
                    hh = r1.tile([4, H], F32, tag="hh")
                    nc.scalar.activation(
                        out=hh[:], in_=a_t[:],
                        func=mybir.ActivationFunctionType.Tanh,
                    )
                    d_t = r1.tile([4, H], F32, tag="d")
                    nc.gpsimd.tensor_tensor(d_t[:], h_sb[:], hh[:], SUB)
                    e_t = r1.tile([4, H], F32, tag="e")
                    nc.vector.tensor_tensor(e_t[:], zr[:, 0:H], d_t[:], MULT)
                    nc.gpsimd.tensor_tensor(h_sb[:], hh[:], e_t[:], ADD)

                    # transpose h -> hst[:, :, b*T + t]
                    tp = tps2.tile([128, KC, 4], F32, tag="tp2")
                    for k in range(KC):
                        nc.tensor.transpose(
                            tp[:, k, :], h_sb[:, 128 * k : 128 * (k + 1)],
                            id4_sb[:],
                        )
                    nc.vector.tensor_copy(hst[:, :, t::T_], tp[:])
                    nc.scalar.copy(hstr[:, :, t::T_], tp[:])

            # ---------------- phase 3: projection ----------------
            with (
                tc.tile_pool(name="wop", bufs=2) as wop,
                tc.tile_pool(name="post", bufs=4) as post,
                tc.tile_pool(name="borp", bufs=2) as borp,
                tc.tile_pool(name="prps", bufs=8, space="PSUM") as prps,
            ):
                wor = wo[:].rearrange("(k p) v -> p k v", p=128)
                for v in range(NVB):
                    vs = slice(VBLK * v, VBLK * (v + 1))
                    woc = wop.tile([128, KC, VBLK], DT_PROJ, tag="wo")
                    nc.sync.dma_start(out=woc, in_=c_proj(wor[:, :, vs]))
                    borr = borp.tile([128, VBLK], F32, tag="bor")
                    nc.sync.dma_start(
                        out=borr,
                        in_=bass.AP(
                            tensor=bo, offset=VBLK * v, ap=[[0, 128], [1, VBLK]]
                        ),
                    )
                    for b in range(BS):
                        pr = prps.tile([128, VBLK], F32, tag="pr")
                        for k in range(KC):
                            nc.tensor.matmul(
                                pr[:],
                                hstr[:, k, T_ * b : T_ * (b + 1)],
                                woc[:, k, :],
                                start=(k == 0), stop=(k == KC - 1),
                            )
                        o1 = post.tile([128, VBLK], F32, tag="o1")
                        nc.vector.tensor_tensor(o1[:], pr[:], borr[:], ADD)
                        o2 = post.tile([128, VBLK], F32, tag="o2")
                        nc.scalar.mul(o2[:], o1[:], actm[:, b : b + 1])
                        nc.scalar.dma_start(out=out[b, :, vs], in_=o2[:])

    nc.compile()
    return nc


_CACHED = {}


def _get_kernel():
    if "nc" not in _CACHED:
        _CACHED["nc"] = build_kernel()
    return _CACHED["nc"]


def host_consts():
    id4 = np.eye(4, dtype=np.float32)
    id128 = np.eye(128, dtype=np.float32)
    import ml_dtypes
    sels = np.zeros((128, 8, 4), dtype=ml_dtypes.bfloat16)
    for g in range(4):
        for j in range(8):
            for m in range(4):
                sels[32 * g + 4 * j + m, j, m] = 1.0
    onesc = np.ones((128, 128), dtype=np.float32)
    return {"id4": id4, "id128": id128, "sels": sels, "onesc": onesc}


def make_in_maps(
    encoder_outputs, encoder_state, y, mask, emb, W_in, b_in, U, b_rec, Wo, bo,
    n_cores=N_CORES,
):
    consts = host_consts()
    h3 = 3 * H
    in_maps = []
    bs = encoder_outputs.shape[0] // n_cores
    for c in range(n_cores):
        rows = slice(bs * c, bs * (c + 1))
        in_maps.append(
            {
                "enc_out": np.ascontiguousarray(encoder_outputs[rows], np.float32),
                "enc_st": np.ascontiguousarray(encoder_state[rows], np.float32),
                "y": np.ascontiguousarray(y[rows], np.int32),
                "mask": np.ascontiguousarray(mask[rows], np.int32),
                "emb": np.ascontiguousarray(emb, np.float32),
                "w_in": np.ascontiguousarray(W_in, np.float32),
                "b_in": np.ascontiguousarray(b_in, np.float32).reshape(1, h3),
                "u_w": np.ascontiguousarray(U, np.float32),
                "b_rec": np.ascontiguousarray(b_rec, np.float32).reshape(1, h3),
                "wo": np.ascontiguousarray(Wo, np.float32),
                "bo": np.ascontiguousarray(bo, np.float32).reshape(1, -1),
                **consts,
            }
        )
    return in_maps


def kernel(
    encoder_outputs, encoder_state, y, mask, emb, W_in, b_in, U, b_rec, Wo, bo
):
    if np.any(np.asarray(b_rec).reshape(-1)[2 * H :]):
        raise NotImplementedError("nonzero b_rec_h not supported")
    nc = _get_kernel()
    in_maps = make_in_maps(
        encoder_outputs, encoder_state, y, mask, emb, W_in, b_in, U, b_rec,
        Wo, bo,
    )
    res = run_bass_kernel_spmd(nc, in_maps, core_ids=list(range(N_CORES)))
    outs = [res.results[c]["out"] for c in range(N_CORES)]
    return np.concatenate(outs, axis=0).astype(np.float32)
